# revision 9
# baseline (speedup 1.0000x reference)
"""3D Gaussian blur (kernel_size=5, sigma=1.0) on (2,1,192,256,256) f32,
distributed over 8 Trainium2 NeuronCores.

The reference kernel factors exactly: g[i,j,l] = u[i]*v[l] (indep. of j),
so the 3D conv is separable into three 1D convs: Gaussian along D, box
along H, Gaussian along W.

Sharding: data-parallel over (batch, D-slab): 8 cores = 2 batches x 4 slabs
of 48 output slices each; each core receives its slab plus a 2-slice halo
(zero slices at batch edges), i.e. input [52, 256, 256].

Per-core kernel (Bass/Tile), fully separable (v2):
  stage D (Vector engine): 5-tap Gaussian along D as a chain of 4 fused
    scalar_tensor_tensor ops (out = (in0*s) + in1), all bf16, producing
    X' = (sum_i aD[i] X[s+i]) / aD[0]; the 1/aD[0] is folded into the
    pass-A band matrix.
  pass A (TensorE, per slice): banded matmul out = X'.T @ Bh fusing the
    H box conv with an H<->W transpose (h-major -> w-major), bf16.
    PSUM f32 evacuated to bf16 Y by the Scalar engine.
  pass B (TensorE, per slice): banded matmul fusing the W Gaussian conv
    and the transpose back to h-major. PSUM f32 evacuated to bf16 by the
    GpSimd engine; output DMA'd out as bf16 and upcast on host.
Band matrices encode zero-padding at the edges natively.
"""
import numpy as np
import ml_dtypes

import concourse.bacc as bacc
import concourse.tile as tile
from concourse import mybir
from concourse.bass_utils import run_bass_kernel_spmd

B = 2          # batch
D = 192        # depth
HW = 256       # height = width
SLAB = 48      # output slices per core
DIN = SLAB + 4  # input slices per core (2-slice halo each side)
NB = 130       # band-split matmul N (128 + 2*2 halo)
P = 128
N_CORES = 8
GRP = 8        # slices per D-conv group / output DMA group
NPAIR = SLAB // 2
LAG = 2        # pairs between pass-A evac and pass-B consumption

F32 = mybir.dt.float32
BF16 = mybir.dt.bfloat16


def _taps():
    c = np.arange(5, dtype=np.float64) - 2
    u = np.exp(-c * c / 2.0)   # D-axis Gaussian (sigma=1)
    v = np.exp(-c * c)         # W-axis Gaussian (sigma^2=1/2)
    aD = u / u.sum()
    bW = (v / v.sum()).astype(np.float32)
    box = np.full(5, 0.2, dtype=np.float64)
    return aD, box, bW


def _band(rows, cols, roff, coff, taps):
    """M[r, c] = taps[(r+roff) - (c+coff) + 2] where |diff| <= 2, else 0."""
    m = np.zeros((rows, cols), dtype=np.float32)
    for r in range(rows):
        g = r + roff
        for c in range(cols):
            d = g - (c + coff)
            if -2 <= d <= 2:
                m[r, c] = taps[d + 2]
    return m


def _const_tensors():
    aD, box, bW = _taps()
    # fold the D-chain normalizer aD[0] into the H band taps
    bh_taps = (box * aD[0]).astype(np.float32)
    bh = np.stack([
        _band(P, NB, 0, 0, bh_taps),
        _band(P, NB, P, HW - NB, bh_taps),
    ])  # [2, 128, 130]
    bw = np.stack([
        _band(P, NB, 0, 0, bW),
        _band(P, NB, P, HW - NB, bW),
    ])  # [2, 128, 130]
    return bh.astype(ml_dtypes.bfloat16), bw.astype(ml_dtypes.bfloat16)


def _chain_ratios():
    aD, _, _ = _taps()
    # chain c1=(X0*r1)+X1; c2=(c1*r2)+X2; c3=(c2*r3)+X3; c4=(c3*r4)+X4
    # yields (sum aD[i] X[i]) / aD[0]
    r1 = aD[0] / aD[1]
    r2 = aD[1] / aD[2]
    r3 = aD[2] / aD[1]
    r4 = aD[1] / aD[0]
    return float(r1), float(r2), float(r3), float(r4)


def _build_nc():
    nc = bacc.Bacc("TRN2", target_bir_lowering=False, debug=False,
                   num_devices=N_CORES)
    # input pre-swizzled on host: x[p, s, hh, w] = slab[s, hh*128+p, w]
    x_d = nc.declare_dram_parameter("x", [P, DIN, 2, HW], BF16, isOutput=False)
    bh_d = nc.declare_dram_parameter("bh", [2, P, NB], BF16, isOutput=False)
    bw_d = nc.declare_dram_parameter("bw", [2, P, NB], BF16, isOutput=False)
    # output swizzled: out[p, d, hb, w] = slice_d[hb*128+p, w]  (bf16)
    out_d = nc.declare_dram_parameter("out", [P, SLAB, 2, HW], BF16,
                                      isOutput=True)

    r1, r2, r3, r4 = _chain_ratios()
    NCHUNK = (DIN + GRP - 1) // GRP  # 7 chunks: 6x8 + 1x4
    NGRP = SLAB // GRP               # 6 D-conv / output groups

    mult = mybir.AluOpType.mult
    add = mybir.AluOpType.add

    with tile.TileContext(nc) as tc:
        with (
            tc.tile_pool(name="consts", bufs=1) as cpool,
            tc.tile_pool(name="xbf", bufs=1) as xpool,
            tc.tile_pool(name="tmp", bufs=2) as tpool,
            tc.tile_pool(name="xp", bufs=3) as xppool,
            tc.tile_pool(name="y", bufs=LAG + 4) as ypool,
            tc.tile_pool(name="osb", bufs=2) as opool,
            tc.tile_pool(name="pa", bufs=2, space="PSUM") as pa_pool,
            tc.tile_pool(name="pb", bufs=2, space="PSUM") as pb_pool,
        ):
            bh_sb = cpool.tile([P, 2 * NB], BF16, tag="bh")
            bw_sb = cpool.tile([P, 2 * NB], BF16, tag="bw")

            # const DMAs on ACT's hwdge queue
            nc.scalar.dma_start(bh_sb[:, 0:NB], bh_d[0])
            nc.scalar.dma_start(bh_sb[:, NB:2 * NB], bh_d[1])
            nc.scalar.dma_start(bw_sb[:, 0:NB], bw_d[0])
            nc.scalar.dma_start(bw_sb[:, NB:2 * NB], bw_d[1])

            # input chunk DMAs (8 slices each, last = 4), alternating queues
            xchunks = []
            for ci in range(NCHUNK):
                st = ci * GRP
                n = min(GRP, DIN - st)
                xc = xpool.tile([P, n, 2, HW], BF16, tag=f"xb{ci}")
                xchunks.append(xc)
                q = nc.sync if ci % 2 == 0 else nc.scalar
                q.dma_start(xc[:], x_d[:, st:st + n])

            def xin(s0, n):
                """AP view of input slices [s0, s0+n) — must lie in one chunk."""
                ci, off = divmod(s0, GRP)
                assert off + n <= xchunks[ci].shape[1]
                return xchunks[ci][:, off:off + n]

            def emit_stt(eng, out_ap, ratio, s0_in1, lo, hi, from_tmp):
                """out[:, lo:hi] = (tmp[:, lo:hi] * ratio) + X[s0_in1+lo : +hi],
                splitting at input-chunk boundaries (in1 side; tmp contiguous).
                """
                done = lo
                while done < hi:
                    s1 = s0_in1 + done
                    room = GRP - (s1 % GRP)
                    take = min(hi - done, room)
                    eng.scalar_tensor_tensor(
                        out_ap[:, done:done + take],
                        from_tmp[:, done:done + take], ratio,
                        xin(s1, take), mult, add)
                    done += take

            xp_tiles = {}

            def emit_chain(g):
                """D-conv for output slices [g*GRP, g*GRP+GRP)."""
                s0 = g * GRP
                t1 = tpool.tile([P, GRP, 2, HW], BF16, tag="t1")
                t2 = tpool.tile([P, GRP, 2, HW], BF16, tag="t2")
                xp = xppool.tile([P, GRP, 2, HW], BF16, tag="xp")
                xp_tiles[g] = xp
                # c1 = (X[s] * r1) + X[s+1]; in0 also splits at boundaries
                done = 0
                while done < GRP:
                    s = s0 + done
                    take = min(GRP - done, GRP - (s % GRP), GRP - ((s + 1) % GRP))
                    take = max(take, 1)
                    nc.vector.scalar_tensor_tensor(
                        t1[:, done:done + take], xin(s, take), r1,
                        xin(s + 1, take), mult, add)
                    done += take
                emit_stt(nc.vector, t2, r2, s0 + 2, 0, GRP, t1)
                emit_stt(nc.vector, t1, r3, s0 + 3, 0, GRP, t2)
                emit_stt(nc.vector, xp, r4, s0 + 4, 0, GRP, t1)

            emit_chain(0)
            emit_chain(1)

            ys = {}
            o_sb = None
            for i in range(NPAIR + LAG):
                if i < NPAIR:
                    g = i // (GRP // 2)
                    if i % (GRP // 2) == 0 and g + 2 < NGRP:
                        emit_chain(g + 2)
                    # pass A: H box conv + transpose -> w-major
                    a_ps = pa_pool.tile([P, 2, 2, HW], F32, tag="aps")
                    for k in range(2):
                        s = 2 * i + k
                        xp = xp_tiles[s // GRP]
                        sl = s % GRP
                        for wblk in range(2):
                            nc.tensor.matmul(
                                a_ps[:, k, wblk, 0:NB],
                                xp[:, sl, 0, wblk * P: wblk * P + P],
                                bh_sb[:, 0:NB],
                                start=wblk == 0, stop=False)
                            nc.tensor.matmul(
                                a_ps[:, k, wblk, HW - NB:HW],
                                xp[:, sl, 1, wblk * P: wblk * P + P],
                                bh_sb[:, NB:2 * NB],
                                start=False, stop=wblk == 1)
                    y2 = ypool.tile([P, 2, 2, HW], BF16, tag="y")
                    ys[i] = y2
                    nc.scalar.copy(y2[:], a_ps[:])

                j = i - LAG
                if not (0 <= j < NPAIR):
                    continue
                # pass B: W gauss conv + transpose back to h-major
                o_ps = pb_pool.tile([P, 2, 2, HW], F32, tag="ops")
                ysrc = ys[j]
                for k in range(2):
                    for kh in range(2):
                        rhs = bw_sb[:, kh * NB:(kh + 1) * NB]
                        col0 = 0 if kh == 0 else HW - NB
                        for hb in range(2):
                            nc.tensor.matmul(
                                o_ps[:, k, hb, col0: col0 + NB],
                                ysrc[:, k, kh, hb * P: hb * P + P],
                                rhs,
                                start=kh == 0 and hb == 0,
                                stop=kh == 1 and hb == 1)
                pg = j % (GRP // 2)
                if pg == 0:
                    o_sb = opool.tile([P, GRP, 2, HW], BF16, tag="osb")
                # evacB split 1:1 between Scalar and DVE (GpSimd can't read PSUM)
                if j % 2 == 0:
                    nc.scalar.copy(o_sb[:, 2 * pg: 2 * pg + 2], o_ps[:])
                else:
                    nc.vector.tensor_copy(o_sb[:, 2 * pg: 2 * pg + 2], o_ps[:])
                if pg == GRP // 2 - 1:
                    g2 = j // (GRP // 2)
                    nc.sync.dma_start(
                        out_d[:, g2 * GRP:(g2 + 1) * GRP], o_sb[:])

    nc.compile()
    return nc


_NC_CACHE = {}


def _get_nc():
    if "nc" not in _NC_CACHE:
        _NC_CACHE["nc"] = _build_nc()
    return _NC_CACHE["nc"]


def kernel(x, kernel_size, _trace=False, _trace_kwargs=None):
    """x: (2, 1, 192, 256, 256) float32; kernel_size: 5. Returns same shape."""
    assert int(kernel_size) == 5, "kernel hardcodes kernel_size=5"
    x = np.asarray(x)
    assert x.shape == (B, 1, D, HW, HW), x.shape
    in_dtype = x.dtype

    nc = _get_nc()
    bh, bw = _const_tensors()

    xp = np.zeros((B, D + 4, HW, HW), dtype=ml_dtypes.bfloat16)
    xp[:, 2:D + 2] = x[:, 0].astype(ml_dtypes.bfloat16)

    in_maps = []
    for c in range(N_CORES):
        b, j = divmod(c, 4)
        shard = xp[b, j * SLAB: j * SLAB + DIN]  # [52, 256, 256]
        sw = np.ascontiguousarray(
            shard.reshape(DIN, 2, P, HW).transpose(2, 0, 1, 3))
        in_maps.append({
            "x": sw,
            "bh": bh,
            "bw": bw,
        })

    res = run_bass_kernel_spmd(
        nc, in_maps, core_ids=list(range(N_CORES)),
        trace=_trace, **(_trace_kwargs or {}))

    out = np.empty((B, 1, D, HW, HW), dtype=np.float32)
    for c in range(N_CORES):
        b, j = divmod(c, 4)
        r = res.results[c]["out"]  # [128, 48, 2, 256] bf16
        out[b, 0, j * SLAB:(j + 1) * SLAB] = (
            r.astype(np.float32).transpose(1, 2, 0, 3).reshape(SLAB, HW, HW))

    if _trace:
        kernel._last_result = res
    return out.astype(in_dtype, copy=False)


# revision 12
# speedup vs baseline: 1.2062x; 1.2062x over previous
"""3D Gaussian blur (kernel_size=5, sigma=1.0) on (2,1,192,256,256) f32,
distributed over 8 Trainium2 NeuronCores.

The reference kernel factors exactly: g[i,j,l] = u[i]*v[l] (indep. of j),
so the 3D conv is separable into three 1D convs: Gaussian along D, box
along H, Gaussian along W.

Sharding: data-parallel over (batch, D-slab): 8 cores = 2 batches x 4 slabs
of 48 output slices each; each core receives its slab plus a 2-slice halo
(zero slices at batch edges), i.e. input [52, 256, 256].

Per-core kernel (Bass/Tile), fully separable (v2):
  stage D (Vector engine): 5-tap Gaussian along D as a chain of 4 fused
    scalar_tensor_tensor ops (out = (in0*s) + in1), all bf16, producing
    X' = (sum_i aD[i] X[s+i]) / aD[0]; the 1/aD[0] is folded into the
    pass-A band matrix.
  pass A (TensorE, per slice): banded matmul out = X'.T @ Bh fusing the
    H box conv with an H<->W transpose (h-major -> w-major), bf16.
    PSUM f32 evacuated to bf16 Y by the Scalar engine.
  pass B (TensorE, per slice): banded matmul fusing the W Gaussian conv
    and the transpose back to h-major. PSUM f32 evacuated to bf16 by the
    GpSimd engine; output DMA'd out as bf16 and upcast on host.
Band matrices encode zero-padding at the edges natively.
"""
import numpy as np
import ml_dtypes

import concourse.bacc as bacc
import concourse.tile as tile
from concourse import mybir
from concourse.bass_utils import run_bass_kernel_spmd

B = 2          # batch
D = 192        # depth
HW = 256       # height = width
SLAB = 48      # output slices per core
DIN = SLAB + 4  # input slices per core (2-slice halo each side)
NB = 130       # band-split matmul N (128 + 2*2 halo)
P = 128
N_CORES = 8
GRP = 8        # slices per D-conv group / output DMA group
NPAIR = SLAB // 2
LAG = 2        # pairs between pass-A evac and pass-B consumption

F32 = mybir.dt.float32
BF16 = mybir.dt.bfloat16


def _taps():
    c = np.arange(5, dtype=np.float64) - 2
    u = np.exp(-c * c / 2.0)   # D-axis Gaussian (sigma=1)
    v = np.exp(-c * c)         # W-axis Gaussian (sigma^2=1/2)
    aD = u / u.sum()
    bW = (v / v.sum()).astype(np.float32)
    box = np.full(5, 0.2, dtype=np.float64)
    return aD, box, bW


def _band(rows, cols, roff, coff, taps):
    """M[r, c] = taps[(r+roff) - (c+coff) + 2] where |diff| <= 2, else 0."""
    m = np.zeros((rows, cols), dtype=np.float32)
    for r in range(rows):
        g = r + roff
        for c in range(cols):
            d = g - (c + coff)
            if -2 <= d <= 2:
                m[r, c] = taps[d + 2]
    return m


def _const_tensors():
    aD, box, bW = _taps()
    # fold the D-chain normalizer aD[0] into the H band taps
    bh_taps = (box * aD[0]).astype(np.float32)
    bh = np.stack([
        _band(P, NB, 0, 0, bh_taps),
        _band(P, NB, P, HW - NB, bh_taps),
    ])  # [2, 128, 130]
    bw = np.stack([
        _band(P, NB, 0, 0, bW),
        _band(P, NB, P, HW - NB, bW),
    ])  # [2, 128, 130]
    return bh.astype(ml_dtypes.bfloat16), bw.astype(ml_dtypes.bfloat16)


def _chain_ratios():
    aD, _, _ = _taps()
    # chain c1=(X0*r1)+X1; c2=(c1*r2)+X2; c3=(c2*r3)+X3; c4=(c3*r4)+X4
    # yields (sum aD[i] X[i]) / aD[0]
    r1 = aD[0] / aD[1]
    r2 = aD[1] / aD[2]
    r3 = aD[2] / aD[1]
    r4 = aD[1] / aD[0]
    return float(r1), float(r2), float(r3), float(r4)


def _build_nc():
    nc = bacc.Bacc("TRN2", target_bir_lowering=False, debug=False,
                   num_devices=N_CORES)
    # input pre-swizzled on host: x[p, s, hh, w] = slab[s, hh*128+p, w]
    x_d = nc.declare_dram_parameter("x", [P, DIN, 2, HW], BF16, isOutput=False)
    bh_d = nc.declare_dram_parameter("bh", [2, P, NB], BF16, isOutput=False)
    bw_d = nc.declare_dram_parameter("bw", [2, P, NB], BF16, isOutput=False)
    # output swizzled: out[p, d, hb, w] = slice_d[hb*128+p, w]  (bf16)
    out_d = nc.declare_dram_parameter("out", [P, SLAB, 2, HW], BF16,
                                      isOutput=True)

    r1, r2, r3, r4 = _chain_ratios()
    NCHUNK = (DIN + GRP - 1) // GRP  # 7 chunks: 6x8 + 1x4
    NGRP = SLAB // GRP               # 6 D-conv / output groups

    mult = mybir.AluOpType.mult
    add = mybir.AluOpType.add

    with tile.TileContext(nc) as tc:
        with (
            tc.tile_pool(name="consts", bufs=1) as cpool,
            tc.tile_pool(name="xbf", bufs=1) as xpool,
            tc.tile_pool(name="tmp", bufs=2) as tpool,
            tc.tile_pool(name="xp", bufs=3) as xppool,
            tc.tile_pool(name="y", bufs=LAG + 4) as ypool,
            tc.tile_pool(name="osb", bufs=2) as opool,
            tc.tile_pool(name="pa", bufs=2, space="PSUM") as pa_pool,
            tc.tile_pool(name="pb", bufs=2, space="PSUM") as pb_pool,
        ):
            bh_sb = cpool.tile([P, 2 * NB], BF16, tag="bh")
            bw_sb = cpool.tile([P, 2 * NB], BF16, tag="bw")

            # const DMAs on ACT's hwdge queue
            nc.scalar.dma_start(bh_sb[:, 0:NB], bh_d[0])
            nc.scalar.dma_start(bh_sb[:, NB:2 * NB], bh_d[1])
            nc.scalar.dma_start(bw_sb[:, 0:NB], bw_d[0])
            nc.scalar.dma_start(bw_sb[:, NB:2 * NB], bw_d[1])

            # input chunk DMAs (8 slices each, last = 4), alternating queues
            xchunks = []
            for ci in range(NCHUNK):
                st = ci * GRP
                n = min(GRP, DIN - st)
                xc = xpool.tile([P, n * 2 * HW], BF16, tag=f"xb{ci}")
                xchunks.append(xc)
                q = nc.sync if ci % 2 == 0 else nc.scalar
                q.dma_start(xc[:], x_d[:, st:st + n])

            SL = 2 * HW  # flat elems per slice

            def xin(s0, n):
                """Flat AP view of input slices [s0, s0+n) — in one chunk."""
                ci, off = divmod(s0, GRP)
                assert (off + n) * SL <= xchunks[ci].shape[1]
                return xchunks[ci][:, off * SL:(off + n) * SL]

            def emit_stt(eng, out_ap, ratio, s0_in1, lo, hi, from_tmp):
                """out[:, lo:hi] = (tmp[:, lo:hi] * ratio) + X[s0_in1+lo : +hi],
                splitting at input-chunk boundaries (in1 side; tmp contiguous).
                """
                done = lo
                while done < hi:
                    s1 = s0_in1 + done
                    room = GRP - (s1 % GRP)
                    take = min(hi - done, room)
                    eng.scalar_tensor_tensor(
                        out_ap[:, done * SL:(done + take) * SL],
                        from_tmp[:, done * SL:(done + take) * SL], ratio,
                        xin(s1, take), mult, add)
                    done += take

            xp_tiles = {}

            def emit_chain(g):
                """D-conv for output slices [g*GRP, g*GRP+GRP)."""
                s0 = g * GRP
                t1 = tpool.tile([P, GRP * SL], BF16, tag="t1")
                t2 = tpool.tile([P, GRP * SL], BF16, tag="t2")
                xp = xppool.tile([P, GRP * SL], BF16, tag="xp")
                xp_tiles[g] = xp
                # c1 = (X[s] * r1) + X[s+1]; in0 also splits at boundaries
                done = 0
                while done < GRP:
                    s = s0 + done
                    take = min(GRP - done, GRP - (s % GRP), GRP - ((s + 1) % GRP))
                    take = max(take, 1)
                    nc.vector.scalar_tensor_tensor(
                        t1[:, done * SL:(done + take) * SL], xin(s, take), r1,
                        xin(s + 1, take), mult, add)
                    done += take
                emit_stt(nc.vector, t2, r2, s0 + 2, 0, GRP, t1)
                emit_stt(nc.vector, t1, r3, s0 + 3, 0, GRP, t2)
                emit_stt(nc.vector, xp, r4, s0 + 4, 0, GRP, t1)

            emit_chain(0)
            emit_chain(1)

            ys = {}
            o_sb = None
            for i in range(NPAIR + LAG):
                if i < NPAIR:
                    g = i // (GRP // 2)
                    if i % (GRP // 2) == 0 and g + 2 < NGRP:
                        emit_chain(g + 2)
                    # pass A: H box conv + transpose -> w-major
                    a_ps = pa_pool.tile([P, 2, 2, HW], F32, tag="aps")
                    for k in range(2):
                        s = 2 * i + k
                        xp = xp_tiles[s // GRP]
                        o = (s % GRP) * SL
                        for wblk in range(2):
                            nc.tensor.matmul(
                                a_ps[:, k, wblk, 0:NB],
                                xp[:, o + wblk * P: o + wblk * P + P],
                                bh_sb[:, 0:NB],
                                start=wblk == 0, stop=False)
                            nc.tensor.matmul(
                                a_ps[:, k, wblk, HW - NB:HW],
                                xp[:, o + HW + wblk * P: o + HW + wblk * P + P],
                                bh_sb[:, NB:2 * NB],
                                start=False, stop=wblk == 1)
                    y2 = ypool.tile([P, 2, 2, HW], BF16, tag="y")
                    ys[i] = y2
                    nc.scalar.copy(y2[:], a_ps[:])

                j = i - LAG
                if not (0 <= j < NPAIR):
                    continue
                # pass B: W gauss conv + transpose back to h-major
                o_ps = pb_pool.tile([P, 2, 2, HW], F32, tag="ops")
                ysrc = ys[j]
                for k in range(2):
                    for kh in range(2):
                        rhs = bw_sb[:, kh * NB:(kh + 1) * NB]
                        col0 = 0 if kh == 0 else HW - NB
                        for hb in range(2):
                            nc.tensor.matmul(
                                o_ps[:, k, hb, col0: col0 + NB],
                                ysrc[:, k, kh, hb * P: hb * P + P],
                                rhs,
                                start=kh == 0 and hb == 0,
                                stop=kh == 1 and hb == 1)
                pg = j % (GRP // 2)
                if pg == 0:
                    o_sb = opool.tile([P, GRP, 2, HW], BF16, tag="osb")
                # evacB split 1:1 between Scalar and DVE (GpSimd can't read PSUM)
                if j % 2 == 0:
                    nc.scalar.copy(o_sb[:, 2 * pg: 2 * pg + 2], o_ps[:])
                else:
                    nc.vector.tensor_copy(o_sb[:, 2 * pg: 2 * pg + 2], o_ps[:])
                if pg == GRP // 2 - 1:
                    g2 = j // (GRP // 2)
                    nc.sync.dma_start(
                        out_d[:, g2 * GRP:(g2 + 1) * GRP], o_sb[:])

    nc.compile()
    return nc


_NC_CACHE = {}


def _get_nc():
    if "nc" not in _NC_CACHE:
        _NC_CACHE["nc"] = _build_nc()
    return _NC_CACHE["nc"]


def kernel(x, kernel_size, _trace=False, _trace_kwargs=None):
    """x: (2, 1, 192, 256, 256) float32; kernel_size: 5. Returns same shape."""
    assert int(kernel_size) == 5, "kernel hardcodes kernel_size=5"
    x = np.asarray(x)
    assert x.shape == (B, 1, D, HW, HW), x.shape
    in_dtype = x.dtype

    nc = _get_nc()
    bh, bw = _const_tensors()

    xp = np.zeros((B, D + 4, HW, HW), dtype=ml_dtypes.bfloat16)
    xp[:, 2:D + 2] = x[:, 0].astype(ml_dtypes.bfloat16)

    in_maps = []
    for c in range(N_CORES):
        b, j = divmod(c, 4)
        shard = xp[b, j * SLAB: j * SLAB + DIN]  # [52, 256, 256]
        sw = np.ascontiguousarray(
            shard.reshape(DIN, 2, P, HW).transpose(2, 0, 1, 3))
        in_maps.append({
            "x": sw,
            "bh": bh,
            "bw": bw,
        })

    res = run_bass_kernel_spmd(
        nc, in_maps, core_ids=list(range(N_CORES)),
        trace=_trace, **(_trace_kwargs or {}))

    out = np.empty((B, 1, D, HW, HW), dtype=np.float32)
    for c in range(N_CORES):
        b, j = divmod(c, 4)
        r = res.results[c]["out"]  # [128, 48, 2, 256] bf16
        out[b, 0, j * SLAB:(j + 1) * SLAB] = (
            r.astype(np.float32).transpose(1, 2, 0, 3).reshape(SLAB, HW, HW))

    if _trace:
        kernel._last_result = res
    return out.astype(in_dtype, copy=False)


# revision 13
# speedup vs baseline: 1.6365x; 1.3568x over previous
"""3D Gaussian blur — baseline architecture (D-conv fused into pass B via
PSUM accumulation on TensorE) with bf16 output DMA (host upcast) and the
output stream on the gpsimd DMA queue to overlap with input.
"""
import numpy as np
import ml_dtypes

import concourse.bacc as bacc
import concourse.tile as tile
from concourse import mybir
from concourse.bass_utils import run_bass_kernel_spmd

B = 2
D = 192
HW = 256
SLAB = 48
DIN = SLAB + 4
NB = 130
P = 128
N_CORES = 8

F32 = mybir.dt.float32
BF16 = mybir.dt.bfloat16


def _taps():
    c = np.arange(5, dtype=np.float64) - 2
    u = np.exp(-c * c / 2.0)   # D-axis Gaussian (sigma=1)
    v = np.exp(-c * c)         # W-axis Gaussian (sigma^2=1/2)
    aD = (u / u.sum()).astype(np.float32)
    bW = (v / v.sum()).astype(np.float32)
    box = np.full(5, 0.2, dtype=np.float32)
    return aD, box, bW


def _band(rows, cols, roff, coff, taps):
    m = np.zeros((rows, cols), dtype=np.float32)
    for r in range(rows):
        g = r + roff
        for c in range(cols):
            d = g - (c + coff)
            if -2 <= d <= 2:
                m[r, c] = taps[d + 2]
    return m


def _const_tensors():
    aD, box, bW = _taps()
    bh = np.stack([
        _band(P, NB, 0, 0, box),
        _band(P, NB, P, HW - NB, box),
    ])  # [2, 128, 130]
    bw = np.stack([
        np.stack([
            _band(P, NB, 0, 0, aD[i] * bW),
            _band(P, NB, P, HW - NB, aD[i] * bW),
        ])
        for i in range(5)
    ])  # [5, 2, 128, 130]
    return bh.astype(ml_dtypes.bfloat16), bw.astype(ml_dtypes.bfloat16)


def _build_nc():
    nc = bacc.Bacc("TRN2", target_bir_lowering=False, debug=False,
                   num_devices=N_CORES)
    x_d = nc.declare_dram_parameter("x", [P, DIN, 2, HW], BF16, isOutput=False)
    bh_d = nc.declare_dram_parameter("bh", [2, P, NB], BF16, isOutput=False)
    bw_d = nc.declare_dram_parameter("bw", [5, 2, P, NB], BF16, isOutput=False)
    out_d = nc.declare_dram_parameter("out", [P, SLAB, 2, HW], BF16,
                                      isOutput=True)

    XCHUNKS = [4, 12, 12, 12, 12]
    OCH = 4
    LAG = 3

    with tile.TileContext(nc) as tc:
        with (
            tc.tile_pool(name="consts", bufs=1) as cpool,
            tc.tile_pool(name="xbf", bufs=1) as xpool,
            tc.tile_pool(name="y", bufs=DIN // 2 + 1) as ypool,
            tc.tile_pool(name="osb", bufs=3) as opool,
            tc.tile_pool(name="pa", bufs=2, space="PSUM") as pa_pool,
            tc.tile_pool(name="pb", bufs=2, space="PSUM") as pb_pool,
        ):
            chunk_starts = []
            acc = 0
            for n in XCHUNKS:
                chunk_starts.append(acc)
                acc += n
            assert acc == DIN
            chunk_of = {}
            for ci, (st, n) in enumerate(zip(chunk_starts, XCHUNKS)):
                for s in range(st, st + n):
                    chunk_of[s] = (ci, s - st)

            bh_sb = cpool.tile([P, 2 * NB], BF16, tag="bh")
            bw_sb = cpool.tile([P, 10 * NB], BF16, tag="bw")

            # input chunk DMAs alternating across the two hwdge queues
            xchunks = []
            for ci, (st, n) in enumerate(zip(chunk_starts, XCHUNKS)):
                xc = xpool.tile([P, n, 2, HW], BF16, tag=f"xb{ci}")
                xchunks.append(xc)
                q = nc.sync if ci % 2 == 0 else nc.scalar
                q.dma_start(xc[:], x_d[:, st:st + n])

            # const DMAs on ACT's hwdge queue
            nc.scalar.dma_start(bh_sb[:, 0:NB], bh_d[0])
            nc.scalar.dma_start(bh_sb[:, NB:2 * NB], bh_d[1])
            for i in range(5):
                for k in range(2):
                    j = i * 2 + k
                    nc.scalar.dma_start(bw_sb[:, j * NB:(j + 1) * NB], bw_d[i, k])

            ys2 = []

            def yv(s):
                return ys2[s // 2][:, s % 2]

            a_ps = None
            o_ps = None
            o_sb = None
            for it in range(DIN + 4 + LAG):
                s = it
                if s < DIN:
                    ci, sl = chunk_of[s]
                    x_b = xchunks[ci]
                    # pass A: H box conv + transpose -> w-major
                    if s % 2 == 0:
                        a_ps = pa_pool.tile([P, 2, 2, HW], F32, tag="aps")
                    for wblk in range(2):
                        nc.tensor.matmul(
                            a_ps[:, s % 2, wblk, 0:NB],
                            x_b[:, sl, 0, wblk * P: wblk * P + P],
                            bh_sb[:, 0:NB],
                            start=wblk == 0, stop=False)
                        nc.tensor.matmul(
                            a_ps[:, s % 2, wblk, HW - NB:HW],
                            x_b[:, sl, 1, wblk * P: wblk * P + P],
                            bh_sb[:, NB:2 * NB],
                            start=False, stop=wblk == 1)
                    if s % 2 == 1:
                        y2 = ypool.tile([P, 2, 2, HW], BF16, tag="y")
                        ys2.append(y2)
                        nc.scalar.copy(y2[:], a_ps[:])

                dd = it - 4 - LAG
                if not (0 <= dd < SLAB):
                    continue

                # pass B: W gauss conv (x aD tap) + transpose back
                if dd % 2 == 0:
                    o_ps = pb_pool.tile([P, 2, 2, HW], F32, tag="ops")
                n_mm = 0
                for i in range(5):
                    ysrc = yv(dd + i)
                    for kh in range(2):
                        rhs = bw_sb[:, (i * 2 + kh) * NB:(i * 2 + kh + 1) * NB]
                        for hblk in range(2):
                            col0 = 0 if kh == 0 else HW - NB
                            nc.tensor.matmul(
                                o_ps[:, dd % 2, hblk, col0: col0 + NB],
                                ysrc[:, kh, hblk * P: hblk * P + P],
                                rhs,
                                start=n_mm == 0, stop=n_mm == 19)
                            n_mm += 1

                if dd % OCH == 0:
                    o_sb = opool.tile([P, OCH, 2, HW], BF16, tag="osb")
                if dd % 2 == 1:
                    nc.vector.tensor_copy(
                        o_sb[:, dd % OCH - 1: dd % OCH + 1], o_ps[:])
                if dd % OCH == OCH - 1:
                    nc.gpsimd.dma_start(
                        out_d[:, dd - OCH + 1: dd + 1], o_sb[:])

    nc.compile()
    return nc


_NC_CACHE = {}


def _get_nc():
    if "nc" not in _NC_CACHE:
        _NC_CACHE["nc"] = _build_nc()
    return _NC_CACHE["nc"]


def kernel(x, kernel_size, _trace=False, _trace_kwargs=None):
    """x: (2, 1, 192, 256, 256) float32; kernel_size: 5. Returns same shape."""
    assert int(kernel_size) == 5, "kernel hardcodes kernel_size=5"
    x = np.asarray(x)
    assert x.shape == (B, 1, D, HW, HW), x.shape
    in_dtype = x.dtype

    nc = _get_nc()
    bh, bw = _const_tensors()

    xp = np.zeros((B, D + 4, HW, HW), dtype=ml_dtypes.bfloat16)
    xp[:, 2:D + 2] = x[:, 0].astype(ml_dtypes.bfloat16)

    in_maps = []
    for c in range(N_CORES):
        b, j = divmod(c, 4)
        shard = xp[b, j * SLAB: j * SLAB + DIN]  # [52, 256, 256]
        sw = np.ascontiguousarray(
            shard.reshape(DIN, 2, P, HW).transpose(2, 0, 1, 3))
        in_maps.append({
            "x": sw,
            "bh": bh,
            "bw": bw,
        })

    res = run_bass_kernel_spmd(
        nc, in_maps, core_ids=list(range(N_CORES)),
        trace=_trace, **(_trace_kwargs or {}))

    out = np.empty((B, 1, D, HW, HW), dtype=np.float32)
    for c in range(N_CORES):
        b, j = divmod(c, 4)
        r = res.results[c]["out"]  # [128, 48, 2, 256] bf16
        out[b, 0, j * SLAB:(j + 1) * SLAB] = (
            r.astype(np.float32).transpose(1, 2, 0, 3).reshape(SLAB, HW, HW))

    if _trace:
        kernel._last_result = res
    return out.astype(in_dtype, copy=False)


# revision 14
# speedup vs baseline: 1.6371x; 1.0004x over previous
"""3D Gaussian blur — baseline architecture (D-conv fused into pass B via
PSUM accumulation on TensorE) with bf16 output DMA (host upcast) and the
output stream on the gpsimd DMA queue to overlap with input.
"""
import numpy as np
import ml_dtypes

import concourse.bacc as bacc
import concourse.tile as tile
from concourse import mybir
from concourse.bass_utils import run_bass_kernel_spmd

B = 2
D = 192
HW = 256
SLAB = 48
DIN = SLAB + 4
NB = 130
P = 128
N_CORES = 8

F32 = mybir.dt.float32
BF16 = mybir.dt.bfloat16


def _taps():
    c = np.arange(5, dtype=np.float64) - 2
    u = np.exp(-c * c / 2.0)   # D-axis Gaussian (sigma=1)
    v = np.exp(-c * c)         # W-axis Gaussian (sigma^2=1/2)
    aD = (u / u.sum()).astype(np.float32)
    bW = (v / v.sum()).astype(np.float32)
    box = np.full(5, 0.2, dtype=np.float32)
    return aD, box, bW


def _band(rows, cols, roff, coff, taps):
    m = np.zeros((rows, cols), dtype=np.float32)
    for r in range(rows):
        g = r + roff
        for c in range(cols):
            d = g - (c + coff)
            if -2 <= d <= 2:
                m[r, c] = taps[d + 2]
    return m


def _const_tensors():
    aD, box, bW = _taps()
    bh = np.stack([
        _band(P, NB, 0, 0, box),
        _band(P, NB, P, HW - NB, box),
    ])  # [2, 128, 130]
    bw = np.stack([
        np.stack([
            _band(P, NB, 0, 0, aD[i] * bW),
            _band(P, NB, P, HW - NB, aD[i] * bW),
        ])
        for i in range(5)
    ])  # [5, 2, 128, 130]
    return bh.astype(ml_dtypes.bfloat16), bw.astype(ml_dtypes.bfloat16)


def _build_nc():
    nc = bacc.Bacc("TRN2", target_bir_lowering=False, debug=False,
                   num_devices=N_CORES)
    x_d = nc.declare_dram_parameter("x", [P, DIN, 2, HW], BF16, isOutput=False)
    bh_d = nc.declare_dram_parameter("bh", [2, P, NB], BF16, isOutput=False)
    bw_d = nc.declare_dram_parameter("bw", [5, 2, P, NB], BF16, isOutput=False)
    out_d = nc.declare_dram_parameter("out", [P, SLAB, 2, HW], BF16,
                                      isOutput=True)

    XCHUNKS = [4, 12, 12, 12, 12]
    OCH = 4
    LAG = 3

    with tile.TileContext(nc) as tc:
        with (
            tc.tile_pool(name="consts", bufs=1) as cpool,
            tc.tile_pool(name="xbf", bufs=1) as xpool,
            tc.tile_pool(name="y", bufs=DIN // 2 + 1) as ypool,
            tc.tile_pool(name="osb", bufs=3) as opool,
            tc.tile_pool(name="pa", bufs=2, space="PSUM") as pa_pool,
            tc.tile_pool(name="pb", bufs=2, space="PSUM") as pb_pool,
        ):
            chunk_starts = []
            acc = 0
            for n in XCHUNKS:
                chunk_starts.append(acc)
                acc += n
            assert acc == DIN
            chunk_of = {}
            for ci, (st, n) in enumerate(zip(chunk_starts, XCHUNKS)):
                for s in range(st, st + n):
                    chunk_of[s] = (ci, s - st)

            bh_sb = cpool.tile([P, 2 * NB], BF16, tag="bh")
            bw_sb = cpool.tile([P, 10 * NB], BF16, tag="bw")

            # input chunk DMAs alternating across the two hwdge queues
            xchunks = []
            for ci, (st, n) in enumerate(zip(chunk_starts, XCHUNKS)):
                xc = xpool.tile([P, n, 2, HW], BF16, tag=f"xb{ci}")
                xchunks.append(xc)
                nc.sync.dma_start(xc[:], x_d[:, st:st + n])

            # const DMAs on ACT's hwdge queue
            nc.scalar.dma_start(bh_sb[:, 0:NB], bh_d[0])
            nc.scalar.dma_start(bh_sb[:, NB:2 * NB], bh_d[1])
            for i in range(5):
                for k in range(2):
                    j = i * 2 + k
                    nc.scalar.dma_start(bw_sb[:, j * NB:(j + 1) * NB], bw_d[i, k])

            probe = cpool.tile([P, 3 * 4096], BF16, tag="probe")
            probe_emitted = [False]

            ys2 = []

            def yv(s):
                return ys2[s // 2][:, s % 2]

            a_ps = None
            o_ps = None
            o_sb = None
            for it in range(DIN + 4 + LAG):
                s = it
                if s < DIN:
                    ci, sl = chunk_of[s]
                    x_b = xchunks[ci]
                    # pass A: H box conv + transpose -> w-major
                    if s % 2 == 0:
                        a_ps = pa_pool.tile([P, 2, 2, HW], F32, tag="aps")
                    for wblk in range(2):
                        nc.tensor.matmul(
                            a_ps[:, s % 2, wblk, 0:NB],
                            x_b[:, sl, 0, wblk * P: wblk * P + P],
                            bh_sb[:, 0:NB],
                            start=wblk == 0, stop=False)
                        nc.tensor.matmul(
                            a_ps[:, s % 2, wblk, HW - NB:HW],
                            x_b[:, sl, 1, wblk * P: wblk * P + P],
                            bh_sb[:, NB:2 * NB],
                            start=False, stop=wblk == 1)
                    if s % 2 == 1:
                        y2 = ypool.tile([P, 2, 2, HW], BF16, tag="y")
                        ys2.append(y2)
                        nc.scalar.copy(y2[:], a_ps[:])

                dd = it - 4 - LAG
                if dd == 20 and not probe_emitted[0]:
                    probe_emitted[0] = True
                    madd = mybir.AluOpType.add
                    nc.vector.tensor_tensor(
                        probe[:, 2 * 4096:3 * 4096], probe[:, 0:4096],
                        probe[:, 4096:2 * 4096], madd)
                    nc.vector.tensor_tensor(
                        probe[:, 0:4096], probe[:, 4096:2 * 4096],
                        probe[:, 2 * 4096:3 * 4096], madd)
                if not (0 <= dd < SLAB):
                    continue

                # pass B: W gauss conv (x aD tap) + transpose back
                if dd % 2 == 0:
                    o_ps = pb_pool.tile([P, 2, 2, HW], F32, tag="ops")
                n_mm = 0
                for i in range(5):
                    ysrc = yv(dd + i)
                    for kh in range(2):
                        rhs = bw_sb[:, (i * 2 + kh) * NB:(i * 2 + kh + 1) * NB]
                        for hblk in range(2):
                            col0 = 0 if kh == 0 else HW - NB
                            nc.tensor.matmul(
                                o_ps[:, dd % 2, hblk, col0: col0 + NB],
                                ysrc[:, kh, hblk * P: hblk * P + P],
                                rhs,
                                start=n_mm == 0, stop=n_mm == 19)
                            n_mm += 1

                if dd % OCH == 0:
                    o_sb = opool.tile([P, OCH, 2, HW], BF16, tag="osb")
                if dd % 2 == 1:
                    nc.vector.tensor_copy(
                        o_sb[:, dd % OCH - 1: dd % OCH + 1], o_ps[:])
                if dd % OCH == OCH - 1:
                    nc.scalar.dma_start(
                        out_d[:, dd - OCH + 1: dd + 1], o_sb[:])

    nc.compile()
    return nc


_NC_CACHE = {}


def _get_nc():
    if "nc" not in _NC_CACHE:
        _NC_CACHE["nc"] = _build_nc()
    return _NC_CACHE["nc"]


def kernel(x, kernel_size, _trace=False, _trace_kwargs=None):
    """x: (2, 1, 192, 256, 256) float32; kernel_size: 5. Returns same shape."""
    assert int(kernel_size) == 5, "kernel hardcodes kernel_size=5"
    x = np.asarray(x)
    assert x.shape == (B, 1, D, HW, HW), x.shape
    in_dtype = x.dtype

    nc = _get_nc()
    bh, bw = _const_tensors()

    xp = np.zeros((B, D + 4, HW, HW), dtype=ml_dtypes.bfloat16)
    xp[:, 2:D + 2] = x[:, 0].astype(ml_dtypes.bfloat16)

    in_maps = []
    for c in range(N_CORES):
        b, j = divmod(c, 4)
        shard = xp[b, j * SLAB: j * SLAB + DIN]  # [52, 256, 256]
        sw = np.ascontiguousarray(
            shard.reshape(DIN, 2, P, HW).transpose(2, 0, 1, 3))
        in_maps.append({
            "x": sw,
            "bh": bh,
            "bw": bw,
        })

    res = run_bass_kernel_spmd(
        nc, in_maps, core_ids=list(range(N_CORES)),
        trace=_trace, **(_trace_kwargs or {}))

    out = np.empty((B, 1, D, HW, HW), dtype=np.float32)
    for c in range(N_CORES):
        b, j = divmod(c, 4)
        r = res.results[c]["out"]  # [128, 48, 2, 256] bf16
        out[b, 0, j * SLAB:(j + 1) * SLAB] = (
            r.astype(np.float32).transpose(1, 2, 0, 3).reshape(SLAB, HW, HW))

    if _trace:
        kernel._last_result = res
    return out.astype(in_dtype, copy=False)


# revision 15
# speedup vs baseline: 2.1788x; 1.3309x over previous
"""3D Gaussian blur (kernel_size=5, sigma=1.0) on (2,1,192,256,256) f32,
distributed over 8 Trainium2 NeuronCores.

Separable kernel: G = aD[i] * box[j] * bW[l]. Per-core (Bass/Tile):
  pass A (TensorE): banded matmul fusing the H box conv with the H<->W
    transpose; PSUM f32 evacuated to bf16 Y pairs by the Scalar engine.
  U/V pre-adds (Vector engine, 2x-rate bf16 tensor_tensor): U[t] =
    Y[t]+Y[t+4], V[t] = Y[t+1]+Y[t+3] exploit the symmetric D taps
    [a0,a1,a2,a1,a0], cutting pass B from 5 to 3 tap variants.
  pass B (TensorE): per output slice, 3 variants x 4 banded matmuls
    (U x a0*bW, V x a1*bW, Y[t+2] x a2*bW) accumulate the W Gaussian
    conv + D conv + transpose back in PSUM; evacuated to bf16 (split
    Scalar/Vector) and DMA'd out as bf16 (host upcasts to f32).

Sharding: 8 cores = 2 batches x 4 D-slabs of 48 output slices; each core
gets its slab + 2-slice halo -> input [52, 256, 256] bf16.
"""
import numpy as np
import ml_dtypes

import concourse.bacc as bacc
import concourse.tile as tile
from concourse import mybir
from concourse.bass_utils import run_bass_kernel_spmd

B = 2
D = 192
HW = 256
SLAB = 48
DIN = SLAB + 4
NB = 130
P = 128
N_CORES = 8

F32 = mybir.dt.float32
BF16 = mybir.dt.bfloat16


def _taps():
    c = np.arange(5, dtype=np.float64) - 2
    u = np.exp(-c * c / 2.0)   # D-axis Gaussian (sigma=1)
    v = np.exp(-c * c)         # W-axis Gaussian (sigma^2=1/2)
    aD = (u / u.sum()).astype(np.float32)
    bW = (v / v.sum()).astype(np.float32)
    box = np.full(5, 0.2, dtype=np.float32)
    return aD, box, bW


def _band(rows, cols, roff, coff, taps):
    m = np.zeros((rows, cols), dtype=np.float32)
    for r in range(rows):
        g = r + roff
        for c in range(cols):
            d = g - (c + coff)
            if -2 <= d <= 2:
                m[r, c] = taps[d + 2]
    return m


def _const_tensors():
    aD, box, bW = _taps()
    bh = np.stack([
        _band(P, NB, 0, 0, box),
        _band(P, NB, P, HW - NB, box),
    ])  # [2, 128, 130]
    bw = np.stack([
        np.stack([
            _band(P, NB, 0, 0, aD[i] * bW),
            _band(P, NB, P, HW - NB, aD[i] * bW),
        ])
        for i in range(3)
    ])  # [3, 2, 128, 130]  (tap variants for U, V, center)
    return bh.astype(ml_dtypes.bfloat16), bw.astype(ml_dtypes.bfloat16)


def _build_nc():
    nc = bacc.Bacc("TRN2", target_bir_lowering=False, debug=False,
                   num_devices=N_CORES)
    x_d = nc.declare_dram_parameter("x", [P, DIN, 2, HW], BF16, isOutput=False)
    bh_d = nc.declare_dram_parameter("bh", [2, P, NB], BF16, isOutput=False)
    bw_d = nc.declare_dram_parameter("bw", [3, 2, P, NB], BF16, isOutput=False)
    out_d = nc.declare_dram_parameter("out", [P, SLAB, 2, HW], BF16,
                                      isOutput=True)

    XCHUNKS = [4, 8, 8, 8, 8, 8, 8]
    OCH = 2
    LAG = 3
    madd = mybir.AluOpType.add

    with tile.TileContext(nc) as tc:
        with (
            tc.tile_pool(name="consts", bufs=1) as cpool,
            tc.tile_pool(name="xbf", bufs=1) as xpool,
            tc.tile_pool(name="y", bufs=DIN // 2 + 1) as ypool,
            tc.tile_pool(name="u", bufs=4) as upool,
            tc.tile_pool(name="v", bufs=4) as vpool,
            tc.tile_pool(name="osb", bufs=4) as opool,
            tc.tile_pool(name="pa", bufs=2, space="PSUM") as pa_pool,
            tc.tile_pool(name="pb", bufs=2, space="PSUM") as pb_pool,
        ):
            chunk_starts = []
            acc = 0
            for n in XCHUNKS:
                chunk_starts.append(acc)
                acc += n
            assert acc == DIN
            chunk_of = {}
            for ci, (st, n) in enumerate(zip(chunk_starts, XCHUNKS)):
                for s in range(st, st + n):
                    chunk_of[s] = (ci, s - st)

            bh_sb = cpool.tile([P, 2 * NB], BF16, tag="bh")
            bw_sb = cpool.tile([P, 6 * NB], BF16, tag="bw")

            # consts first on ACT's queue (tiny, do not delay evacs)
            nc.scalar.dma_start(bh_sb[:, 0:NB], bh_d[0])
            nc.scalar.dma_start(bh_sb[:, NB:2 * NB], bh_d[1])
            for i in range(3):
                for k in range(2):
                    j = i * 2 + k
                    nc.scalar.dma_start(bw_sb[:, j * NB:(j + 1) * NB], bw_d[i, k])

            # input chunks alternate across the two hw queues for 2x ramp
            xchunks = []
            for ci, (st, n) in enumerate(zip(chunk_starts, XCHUNKS)):
                xc = xpool.tile([P, n, 2, HW], BF16, tag=f"xb{ci}")
                xchunks.append(xc)
                q = nc.sync if ci % 2 == 0 else nc.scalar
                q.dma_start(xc[:], x_d[:, st:st + n])

            ys2 = []
            u2 = {}
            v2 = {}

            def yv(s):
                return ys2[s // 2][:, s % 2]

            a_ps = None
            o_ps = None
            o_sb = None
            for it in range(DIN + 4 + LAG):
                s = it
                if s < DIN:
                    ci, sl = chunk_of[s]
                    x_b = xchunks[ci]
                    # pass A: H box conv + transpose -> w-major
                    if s % 2 == 0:
                        a_ps = pa_pool.tile([P, 2, 2, HW], F32, tag="aps")
                    for wblk in range(2):
                        nc.tensor.matmul(
                            a_ps[:, s % 2, wblk, 0:NB],
                            x_b[:, sl, 0, wblk * P: wblk * P + P],
                            bh_sb[:, 0:NB],
                            start=wblk == 0, stop=False)
                        nc.tensor.matmul(
                            a_ps[:, s % 2, wblk, HW - NB:HW],
                            x_b[:, sl, 1, wblk * P: wblk * P + P],
                            bh_sb[:, NB:2 * NB],
                            start=False, stop=wblk == 1)
                    if s % 2 == 1:
                        p = s // 2
                        y2 = ypool.tile([P, 2, 2, HW], BF16, tag="y")
                        ys2.append(y2)
                        nc.scalar.copy(y2[:], a_ps[:])
                        # U/V pre-adds for output pair k = p - 2 (DVE, 2x bf16)
                        k = p - 2
                        if 0 <= k < SLAB // 2:
                            u = upool.tile([P, 2, 2, HW], BF16, tag="u")
                            v = vpool.tile([P, 2, 2, HW], BF16, tag="v")
                            u2[k] = u
                            v2[k] = v
                            # U[2k]=Y[2k]+Y[2k+4]; U[2k+1]=Y[2k+1]+Y[2k+5]
                            nc.vector.tensor_tensor(
                                u[:], ys2[k][:], ys2[k + 2][:], madd)
                            # V[2k]=Y[2k+1]+Y[2k+3]; V[2k+1]=Y[2k+2]+Y[2k+4]
                            nc.vector.tensor_tensor(
                                v[:, 0], ys2[k][:, 1], ys2[k + 1][:, 1], madd)
                            nc.vector.tensor_tensor(
                                v[:, 1], ys2[k + 1][:, 0], ys2[k + 2][:, 0],
                                madd)

                dd = it - 4 - LAG
                if not (0 <= dd < SLAB):
                    continue

                # pass B: 3 tap variants x 4 banded matmuls, PSUM accumulate
                if dd % 2 == 0:
                    o_ps = pb_pool.tile([P, 2, 2, HW], F32, tag="ops")
                k = dd // 2
                srcs = (u2[k][:, dd % 2], v2[k][:, dd % 2], yv(dd + 2))
                n_mm = 0
                for i in range(3):
                    ysrc = srcs[i]
                    for kh in range(2):
                        rhs = bw_sb[:, (i * 2 + kh) * NB:(i * 2 + kh + 1) * NB]
                        col0 = 0 if kh == 0 else HW - NB
                        for hblk in range(2):
                            nc.tensor.matmul(
                                o_ps[:, dd % 2, hblk, col0: col0 + NB],
                                ysrc[:, kh, hblk * P: hblk * P + P],
                                rhs,
                                start=n_mm == 0, stop=n_mm == 11)
                            n_mm += 1

                if dd % OCH == 0:
                    o_sb = opool.tile([P, OCH, 2, HW], BF16, tag="osb")
                if dd % 2 == 1:
                    # evacB alternates Scalar / DVE
                    if (dd // 2) % 2 == 0:
                        nc.scalar.copy(o_sb[:], o_ps[:])
                    else:
                        nc.vector.tensor_copy(o_sb[:], o_ps[:])
                if dd % OCH == OCH - 1:
                    g2 = dd // OCH
                    q = nc.sync if g2 % 2 == 0 else nc.scalar
                    q.dma_start(out_d[:, dd - OCH + 1: dd + 1], o_sb[:])

    nc.compile()
    return nc


_NC_CACHE = {}


def _get_nc():
    if "nc" not in _NC_CACHE:
        _NC_CACHE["nc"] = _build_nc()
    return _NC_CACHE["nc"]


def kernel(x, kernel_size, _trace=False, _trace_kwargs=None):
    """x: (2, 1, 192, 256, 256) float32; kernel_size: 5. Returns same shape."""
    assert int(kernel_size) == 5, "kernel hardcodes kernel_size=5"
    x = np.asarray(x)
    assert x.shape == (B, 1, D, HW, HW), x.shape
    in_dtype = x.dtype

    nc = _get_nc()
    bh, bw = _const_tensors()

    xp = np.zeros((B, D + 4, HW, HW), dtype=ml_dtypes.bfloat16)
    xp[:, 2:D + 2] = x[:, 0].astype(ml_dtypes.bfloat16)

    in_maps = []
    for c in range(N_CORES):
        b, j = divmod(c, 4)
        shard = xp[b, j * SLAB: j * SLAB + DIN]  # [52, 256, 256]
        sw = np.ascontiguousarray(
            shard.reshape(DIN, 2, P, HW).transpose(2, 0, 1, 3))
        in_maps.append({
            "x": sw,
            "bh": bh,
            "bw": bw,
        })

    res = run_bass_kernel_spmd(
        nc, in_maps, core_ids=list(range(N_CORES)),
        trace=_trace, **(_trace_kwargs or {}))

    out = np.empty((B, 1, D, HW, HW), dtype=np.float32)
    for c in range(N_CORES):
        b, j = divmod(c, 4)
        r = res.results[c]["out"]  # [128, 48, 2, 256] bf16
        out[b, 0, j * SLAB:(j + 1) * SLAB] = (
            r.astype(np.float32).transpose(1, 2, 0, 3).reshape(SLAB, HW, HW))

    if _trace:
        kernel._last_result = res
    return out.astype(in_dtype, copy=False)
